# revision 19
# baseline (speedup 1.0000x reference)
"""Trainium2 Bass kernel for nn_MultiHeadCrossAttention (B=4, T=1024, E=1024, H=16).

Sharding: 8 fully independent shards (output stream s, batch b), zero
cross-core communication. Core c<4 computes stream-1 batch c; core c>=4
stream-2 batch c-4.

Per-core kernel (activations transposed, feature-on-partition):
  Q^T = Wq^T.T @ B^T, K^T = Wk^T.T @ A^T   (f16 matmuls, K=1024)
  V   = A^T.T @ Wv^T                        (natural layout, f16, +ones col)
  per head pair (2m, 2m+1):
    S^T[j,i] = K^T.T @ Q^T (K=64 halves);  P^T = exp(S^T/8) on ACT (f16)
    O'^T = V.T @ P^T accumulated over j-chunks; row 64 = rowsums
    recip = reciprocal_approx_fast (DVE), partition_broadcast (GPSIMD),
    O^T = O'^T * bcast (DVE)
  Z^T = Wout^T.T @ O^T  (accumulate over head chunks), f16 out DMA.

Weights are packed m-major on host so each output-block's stationary
weights arrive in a single contiguous DMA; activations are loaded in
per-chunk tiles so the first matmul can start ~1.5us in.
"""

import os
import sys

sys.path.insert(0, "/opt/trn_rl_repo")

import numpy as np
from contextlib import ExitStack

import concourse.bass as bass
import concourse.mybir as mybir
import concourse.tile as tile
from concourse import bacc
from concourse import bass_utils

B, T, E, H = 4, 1024, 1024, 16
D = E // H            # 64
NC = E // 128         # 8 chunks of 128
NIC = T // 512        # 2 free-dim chunks of 512
N_CORES = 8

F32 = mybir.dt.float32
F16 = mybir.dt.float16

_NC_CACHE = {}
LAST_RESULTS = {}


def _build():
    nc = bacc.Bacc("TRN2", target_bir_lowering=False, debug=False,
                   enable_asserts=False, num_devices=N_CORES)
    a_t = nc.dram_tensor("a_t", (E, T), F16, kind="ExternalInput").ap()
    b_t = nc.dram_tensor("b_t", (E, T), F16, kind="ExternalInput").ap()
    wq_p = nc.dram_tensor("wq_p", (E, E), F16, kind="ExternalInput").ap()
    wk_p = nc.dram_tensor("wk_p", (E, E), F16, kind="ExternalInput").ap()
    wv_t = nc.dram_tensor("wv_t", (E, E), F16, kind="ExternalInput").ap()
    wout_p = nc.dram_tensor("wout_p", (E, E), F16, kind="ExternalInput").ap()
    z_t = nc.dram_tensor("z_t", (E, T), F16, kind="ExternalOutput").ap()

    EXP = mybir.ActivationFunctionType.Exp

    with tile.TileContext(nc) as tc, ExitStack() as ctx:
        big = ctx.enter_context(tc.tile_pool(name="big", bufs=1))
        qt = big.tile([128, NC, T], F16, tag="qt")
        kt = big.tile([128, NC, T], F16, tag="kt")
        v = big.tile([128, NC, H * (D + 1)], F16, tag="v")
        ot = big.tile([128, NC, T], F16, tag="ot")
        wom = [big.tile([128, E], F16, name=f"wom{m}", tag=f"wo{m}") for m in range(NC)]

        # ones column at x=D of each head slice: the O' matmul lands the
        # softmax denominator in PSUM partition 64 (32-aligned, legal AP)
        # with O'^T on partitions 0:64.
        for _m in range(NC):
            nc.vector.memset(
                v[:, _m, :].rearrange("p (h x) -> p h x", x=D + 1)[:, :, D:D + 1], 1.0)

        # S-psum pool spans proj+attention (no pool barrier for the attention
        # matmuls) and closes before Z so zps can reuse its banks as soon as
        # the last exp — rather than the m=7 recip/mul tail — completes.
        phase_ctx = ExitStack()
        spool = phase_ctx.enter_context(tc.tile_pool(name="spool", bufs=2, space="PSUM"))

        # ---------------- Phase P: projections (order Q, V, K) ----------------
        with tc.tile_pool(name="acts", bufs=1) as acts, \
             tc.tile_pool(name="pps", bufs=2, space="PSUM") as pps:
            btc = [acts.tile([128, T], F16, name=f"btc{c}", tag=f"bt{c}") for c in range(NC)]
            atc = [acts.tile([128, T], F16, name=f"atc{c}", tag=f"at{c}") for c in range(NC)]
            wqm = [acts.tile([128, E], F16, name=f"wqm{m}", tag=f"wq{m}") for m in range(NC)]
            wkm = [acts.tile([128, E], F16, name=f"wkm{m}", tag=f"wk{m}") for m in range(NC)]
            wvc = [acts.tile([128, E], F16, name=f"wvc{c}", tag=f"wv{c}") for c in range(NC)]

            # DMA order tracks matmul consumption order. V-proj (at, wv)
            # first so the first matmul group starts after ~2 transfers;
            # V copies then complete during Q-proj so the attention O
            # matmuls never wait on them.
            for c in range(NC):
                nc.sync.dma_start(atc[c][:], a_t[c * 128:(c + 1) * 128, :])
                nc.sync.dma_start(wvc[c][:], wv_t[c * 128:(c + 1) * 128, :])
            nc.sync.dma_start(wqm[0][:], wq_p[0:128, :])
            for c in range(NC):
                nc.sync.dma_start(btc[c][:], b_t[c * 128:(c + 1) * 128, :])
                if c + 1 < NC:
                    nc.sync.dma_start(wqm[c + 1][:], wq_p[(c + 1) * 128:(c + 2) * 128, :])
            for c in range(NC):
                nc.sync.dma_start(wkm[c][:], wk_p[c * 128:(c + 1) * 128, :])
            for m in range(NC):
                nc.sync.dma_start(wom[m][:], wout_p[m * 128:(m + 1) * 128, :])

            # V natural: out[j-chunk][dv] = sum_e at[e, j] * wv[e, dv]
            for m in range(NC):
                ps = pps.tile([128, T], F32, tag="pp")
                for e in range(NC):
                    for ic in range(NIC):
                        nc.tensor.matmul(
                            ps[:, bass.ts(ic, 512)],
                            atc[e][:, bass.ts(m, 128)],
                            wvc[e][:, bass.ts(ic, 512)],
                            start=(e == 0), stop=(e == NC - 1))
                with nc.allow_low_precision(reason="V f16 feeds f16 attention matmuls"):
                    nc.vector.tensor_copy(
                        v[:, m, :].rearrange("p (h x) -> p h x", x=D + 1)[:, :, 0:D],
                        ps[:].rearrange("p (h x) -> p h x", x=D))
            # Q^T: out[dh-chunk m][t] = sum_e w[e, dh] * act[e, t]
            for m in range(NC):
                ps = pps.tile([128, T], F32, tag="pp")
                for e in range(NC):
                    for ic in range(NIC):
                        nc.tensor.matmul(
                            ps[:, bass.ts(ic, 512)],
                            wqm[m][:, bass.ts(e, 128)],
                            btc[e][:, bass.ts(ic, 512)],
                            start=(e == 0), stop=(e == NC - 1))
                nc.scalar.copy(qt[:, m, :], ps[:])
            # K^T last: attention S(m) only needs kt[:, m], so the m-loop can
            # start as soon as each K chunk lands while V is already resident.
            for m in range(NC):
                ps = pps.tile([128, T], F32, tag="pp")
                for e in range(NC):
                    for ic in range(NIC):
                        nc.tensor.matmul(
                            ps[:, bass.ts(ic, 512)],
                            wkm[m][:, bass.ts(e, 128)],
                            atc[e][:, bass.ts(ic, 512)],
                            start=(e == 0), stop=(e == NC - 1))
                nc.scalar.copy(kt[:, m, :], ps[:])

        # ---------------- Phase A: attention per head pair ----------------
        with tc.tile_pool(name="opool", bufs=2, space="PSUM") as opool, \
             tc.tile_pool(name="ptp", bufs=4) as ptp, \
             tc.tile_pool(name="rsp", bufs=2) as rsp, \
             tc.tile_pool(name="bcp", bufs=2) as bcp:
            for m in range(NC):
                ptA = ptp.tile([128, NC, T], F16, tag="pt")
                ptB = ptp.tile([128, NC, T], F16, tag="pt")
                ps_oA = opool.tile([65, T], F32, tag="o")
                ps_oB = opool.tile([65, T], F32, tag="o")
                hA, hB = 2 * m, 2 * m + 1

                def _o_mms(jc):
                    st = dict(start=(jc == 0), stop=(jc == NC - 1))
                    for (ps_o, pt_t, h) in ((ps_oA, ptA, hA), (ps_oB, ptB, hB)):
                        for ic in range(NIC):
                            nc.tensor.matmul(
                                ps_o[:, bass.ts(ic, 512)],
                                v[:, jc, bass.ts(h, D + 1)],
                                pt_t[:, jc, bass.ts(ic, 512)], **st)

                for jc in range(NC):
                    ps_s = spool.tile([128, T], F32, tag="s")
                    ps_sB = spool.tile([128, T], F32, tag="s")
                    for ic in range(NIC):
                        nc.tensor.matmul(
                            ps_s[:, bass.ts(ic, 512)],
                            kt[0:64, m, bass.ts(jc, 128)],
                            qt[0:64, m, bass.ts(ic, 512)],
                            start=True, stop=True)
                    for ic in range(NIC):
                        nc.tensor.matmul(
                            ps_sB[:, bass.ts(ic, 512)],
                            kt[64:128, m, bass.ts(jc, 128)],
                            qt[64:128, m, bass.ts(ic, 512)],
                            start=True, stop=True, tile_position=(64, 0))
                    nc.scalar.activation(ptA[:, jc, :], ps_s[:], EXP, scale=0.125)
                    nc.scalar.activation(ptB[:, jc, :], ps_sB[:], EXP, scale=0.125)
                    if jc >= 2:
                        _o_mms(jc - 2)
                _o_mms(NC - 2)
                _o_mms(NC - 1)

                rs0 = rsp.tile([1, 2 * T], F32, tag="rs0")
                rs = rsp.tile([1, 2 * T], F32, tag="rs")
                bc = bcp.tile([64, 2 * T], F32, tag="bc")
                # standard-op hop to partition 0: custom DVE ops mis-handle
                # non-zero base partitions in their AP lowering. Both heads'
                # rowsums share one row so recip+broadcast run once per pair.
                nc.vector.tensor_copy(rs0[:, 0:T], ps_oA[64:65, :])
                nc.vector.tensor_copy(rs0[:, T:2 * T], ps_oB[64:65, :])
                nc.vector.reciprocal_approx_fast(rs[:], rs0[:])
                nc.gpsimd.partition_broadcast(bc[:], rs[:], channels=64)
                with nc.allow_low_precision(reason="O^T f16 feeds f16 out-proj"):
                    nc.vector.tensor_mul(ot[0:64, m, :], ps_oA[0:64, :], bc[:, 0:T])
                    nc.vector.tensor_mul(ot[64:128, m, :], ps_oB[0:64, :], bc[:, T:2 * T])

        # ---------------- Phase Z: out-projection ----------------
        phase_ctx.close()  # release spool banks (last consumer: final exp)
        with tc.tile_pool(name="zsb", bufs=2) as zsbp, \
             tc.tile_pool(name="zps", bufs=2, space="PSUM") as zps:
            for cc in range(NC):
                ps = zps.tile([128, T], F32, tag="z")
                for m in range(NC):
                    for ic in range(NIC):
                        nc.tensor.matmul(
                            ps[:, bass.ts(ic, 512)],
                            wom[cc][:, bass.ts(m, 128)],
                            ot[:, m, bass.ts(ic, 512)],
                            start=(m == 0), stop=(m == NC - 1))
                zsb = zsbp.tile([128, T], F16, tag="zsb")
                with nc.allow_low_precision(reason="f16 output, converted on host"):
                    nc.vector.tensor_copy(zsb[:], ps[:])
                nc.scalar.dma_start(z_t[cc * 128:(cc + 1) * 128, :], zsb[:])
    nc.compile()
    return nc


def _group_w(wqkv, k):
    """Rows of Wqkv (3E, E) for q/k/v (k=0/1/2), grouped head-major.

    Row index layout: r = di*(3H) + k*H + h  ->  grouped[h*D+di, :].
    """
    w = np.asarray(wqkv, dtype=np.float32).reshape(D, 3, H, E)[:, k]   # [di, h, e]
    return np.ascontiguousarray(w.transpose(1, 0, 2).reshape(E, E))    # [h*D+di, e]


def _pack_mmajor(w_t):
    """[e, d] -> packed so rows m*128+p, cols ec*128+mc = w_t[ec*128+p, m*128+mc].

    One [128, E] DMA then provides the stationary blocks for output-block m:
    packed[m*128:(m+1)*128, e*128:(e+1)*128] == w_t[e*128:(e+1)*128, m*128:(m+1)*128].
    """
    return np.ascontiguousarray(
        w_t.reshape(NC, 128, NC, 128).transpose(2, 1, 0, 3).reshape(E, E))


def kernel(x, y, Wqkv1, Wqkv2, Wout1, Wout2):
    x = np.asarray(x, dtype=np.float32)
    y = np.asarray(y, dtype=np.float32)

    if "nc" not in _NC_CACHE:
        _NC_CACHE["nc"] = _build()
    nc = _NC_CACHE["nc"]

    # weight prep (host): grouped + transposed + m-major packed, f16
    wq1 = _pack_mmajor(_group_w(Wqkv1, 0).T).astype(np.float16)
    wk1 = _pack_mmajor(_group_w(Wqkv1, 1).T).astype(np.float16)
    wv1 = np.ascontiguousarray(_group_w(Wqkv1, 2).T).astype(np.float16)
    wq2 = _pack_mmajor(_group_w(Wqkv2, 0).T).astype(np.float16)
    wk2 = _pack_mmajor(_group_w(Wqkv2, 1).T).astype(np.float16)
    wv2 = np.ascontiguousarray(_group_w(Wqkv2, 2).T).astype(np.float16)
    wo1 = _pack_mmajor(np.asarray(Wout1, dtype=np.float32).T).astype(np.float16)
    wo2 = _pack_mmajor(np.asarray(Wout2, dtype=np.float32).T).astype(np.float16)

    in_maps = []
    for c in range(N_CORES):
        s, b = divmod(c, B)
        if s == 0:
            # stream-1 output: K,V from x via Wqkv1; Q from y via Wqkv2
            a_t, b_t = x[b].T, y[b].T
            wq, wk, wv, wo = wq2, wk1, wv1, wo1
        else:
            a_t, b_t = y[b].T, x[b].T
            wq, wk, wv, wo = wq1, wk2, wv2, wo2
        in_maps.append({
            "a_t": np.ascontiguousarray(a_t).astype(np.float16),
            "b_t": np.ascontiguousarray(b_t).astype(np.float16),
            "wq_p": wq, "wk_p": wk, "wv_t": wv, "wout_p": wo,
        })

    trace = os.environ.get("BASS_KERNEL_TRACE", "0") == "1"
    if trace:
        try:
            from antenv.axon_hooks import get_axon_ntff_profile_hook
            if get_axon_ntff_profile_hook() is None:
                trace = False
        except ImportError:
            trace = False
    ncores = int(os.environ.get("KCORES", str(N_CORES)))
    r = bass_utils.run_bass_kernel_spmd(nc, in_maps[:ncores], core_ids=list(range(ncores)),
                                        trace=trace)
    LAST_RESULTS["exec_time_ns"] = r.exec_time_ns
    LAST_RESULTS["profile_json"] = r.profile_json

    out1 = np.stack([r.results[b]["z_t"].T for b in range(B)]).astype(np.float32)
    out2 = np.stack([r.results[B + b]["z_t"].T for b in range(B)]).astype(np.float32)
    return out1, out2
